# revision 26
# baseline (speedup 1.0000x reference)
"""Trainium2 Bass kernel for nn_NisuyNN_90434831384984.

Math: the reference's stack+reshape makes MLP row (s,t,b) depend only on s
(b in {0,1}) or only on t (b in {2,3}) -- 64 unique rows through the MLP
produce 64 unique 32x32 policy matrices.  The reference applies LeakyReLU
to ALL six layers (including layer 6) before the sigmoid.

Final layout (194.8us HW, vs 257.2us measured baseline):
  - L1..L4: Megatron column-split (512 cols/core); AllGather of the
    transposed fp8 activations after each layer; the 32 K-chunks of each
    layer pack the PE as two concurrent M=64 column groups.
  - W2..W5 are fp8(e4m3) scaled x64 (the /64 rides the activation's
    scale operand, exactly), halving the HBM weight stream so it drains
    inside the ~50us collectives-init barrier window and never contends
    with the latency-critical gather DMAs.
  - L5: column-split, no gather.  L6: row-split against the core's W6
    row-slice (W6 columns host-permuted so output rows are M^T); one bf16
    ReduceScatter sums the partials AND shards the 64 rows 8-per-core.
  - Tail: bias+LeakyReLU+sigmoid+scale into two 128x128 block-diagonal
    bf16 matrices X=diag(M_r^T); the 8-step power iteration becomes
    3 PE squarings (X_{k+1}=Y_k^T X_k, Y=StreamTranspose(X); scale
    cancels in the delta ratios); bv = ones^T X8 lands as a [1,128] row
    so the delta-coefficient tail runs on one partition with strided
    views -- no PE transposes, no DRAM round trip.  Final tiny AllGather
    of per-core [1,128] partial deltas; every core sums the 8 blocks.
  - Dummy warm-spin matmuls after each layer keep the PE HAM clock at
    2.4 GHz across the collective windows; Lrelu/Sigmoid ACT tables are
    preloaded at startup.
"""

import numpy as np

DIM = 128
N = 32
B = 4
H = 4096
NC = 8          # cores
SL = H // NC    # 512 hidden slice per core
OF = N * N      # 1024 output features
R = 64          # unique MLP rows
HR = 32         # rows per stream
KC = 128        # contraction chunk
SLOPE = 0.01
SC = 64.0       # fp8 weight scale (power of two; exact)
WSPIN = 48      # dummy warm matmuls spanning each AG window

_COMPILED = None
LAST_RESULTS = None


def _build_body(nc, tc, tile, mybir, aps):
    f32 = mybir.dt.float32
    bf16 = mybir.dt.bfloat16
    f8 = mybir.dt.float8e4
    AF = mybir.ActivationFunctionType
    ALU = mybir.AluOpType
    AX = mybir.AxisListType
    rg = [list(range(NC))]

    from contextlib import ExitStack
    es = ExitStack()
    cpool = es.enter_context(tc.tile_pool(name="consts", bufs=1))
    wpool = es.enter_context(tc.tile_pool(name="w", bufs=1))
    bpool = es.enter_context(tc.tile_pool(name="b", bufs=1))
    apool = es.enter_context(tc.tile_pool(name="act", bufs=2))
    atp = es.enter_context(tc.tile_pool(name="atT", bufs=2))
    lpool = es.enter_context(tc.tile_pool(name="lhs", bufs=2))
    tailp = es.enter_context(tc.tile_pool(name="tail", bufs=1))
    ps = es.enter_context(tc.tile_pool(name="ps", bufs=2, space="PSUM"))
    pst = es.enter_context(tc.tile_pool(name="pst", bufs=2, space="PSUM"))
    ps6 = es.enter_context(tc.tile_pool(name="ps6", bufs=1, space="PSUM"))
    tps = es.enter_context(tc.tile_pool(name="tps", bufs=2, space="PSUM"))
    dram = es.enter_context(tc.tile_pool(name="dram", bufs=1, space="DRAM"))

    # ---- preload the Lrelu/Sigmoid activation tables during startup ----
    scr0 = cpool.tile([1, 2], f32)
    nc.vector.memset(scr0[:], 0.0)
    scr1 = cpool.tile([1, 2], f32)
    nc.scalar.activation(scr1[:], scr0[:], AF.Lrelu, alpha=SLOPE)
    scr2 = cpool.tile([1, 2], f32)
    nc.scalar.activation(scr2[:], scr0[:], AF.Sigmoid)

    # ---- all input DMAs up front, in consumption order ----
    id64 = cpool.tile([64, 64], bf16)
    nc.sync.dma_start(id64[:], aps["ID64"][:])
    xt = wpool.tile([KC, 2 * R], bf16, tag="xt")
    nc.sync.dma_start(xt[:], aps["XT"][:])
    wts = {}
    bts = {}
    for li in range(1, 7):
        nk = 2 if li == 1 else (H // KC if li < 6 else 4)
        width = OF if li == 6 else SL
        dt = f8 if 2 <= li <= 5 else bf16
        wts[li] = wpool.tile([KC, nk * width], dt, tag=f"w{li}",
                             name=f"wt{li}")
        nc.sync.dma_start(wts[li][:], aps[f"W{li}"][:])
        if li < 6:
            bts[li] = bpool.tile([1, SL], bf16, tag=f"b{li}", name=f"bt{li}")
            nc.sync.dma_start(bts[li][0:1, :], aps[f"b{li}"].unsqueeze(0))

    # ---- constants (scalar queue; gpsimd stays free for CC doorbells) ----
    bias6 = cpool.tile([128, N], f32)
    nc.scalar.dma_start(bias6[:], aps["BIAS6"][:])
    mac = cpool.tile([128, 2], f32)
    nc.scalar.dma_start(mac[:], aps["MAC"][:])
    dmf = cpool.tile([1, 2 * KC], f32)
    nc.scalar.dma_start(dmf[:], aps["DMF"][:])
    ttf = cpool.tile([1, 2 * KC], f32)
    nc.scalar.dma_start(ttf[:], aps["TTF"][:])
    w01r = cpool.tile([1, 8], f32)
    nc.scalar.dma_start(w01r[:], aps["W01R"][:])
    selsf = cpool.tile([1, B * N], f32)
    nc.scalar.dma_start(selsf[:], aps["SELSF"][:])
    seltf = cpool.tile([1, B * N], f32)
    nc.scalar.dma_start(seltf[:], aps["SELTF"][:])
    onesb = cpool.tile([1, R], bf16)
    nc.vector.memset(onesb[:], 1.0)
    ones128 = cpool.tile([128, 1], bf16)
    nc.vector.memset(ones128[:], 1.0)
    x1a = tailp.tile([128, 128], bf16, tag="x1a")
    nc.vector.memset(x1a[:], 0.0)
    x1b = tailp.tile([128, 128], bf16, tag="x1b")
    nc.vector.memset(x1b[:], 0.0)

    def layer_mm(chunks, wt, btile, li):
        """chunks: list of (lhs_ap, global_k).  Returns the [128, SL] psum
        with the two 64-wide column-group partial sums in rows [0:64] and
        [64:128]; bias is accumulated into group 0."""
        pt = ps.tile([2 * R, SL], f32, tag="ps", name=f"pt{li}")
        first = [True, True]
        n_h1 = sum(1 for i in range(len(chunks)) if i % 2 == 1)
        seen_h1 = 0
        for i, (lhs, k) in enumerate(chunks):
            h = i % 2
            if h == 1:
                seen_h1 += 1
            nc.tensor.matmul(
                pt[h * R:(h + 1) * R, :],
                lhs,
                wt[:, k * SL:(k + 1) * SL],
                start=first[h],
                stop=(h == 1 and seen_h1 == n_h1),
                tile_position=(0, h * R),
                skip_group_check=True,
            )
            first[h] = False
        nc.tensor.matmul(
            pt[0:R, :], onesb[0:1, :], btile[0:1, :],
            start=first[0], stop=True, tile_position=(0, 0),
            skip_group_check=True,
        )
        return pt

    def act_transpose(pt, li, scale):
        """psum halves -> z -> LeakyReLU (fp8 unscale folded into the
        activation scale) -> bf16 -> transposed fp8 att tile."""
        z0 = apool.tile([R, SL], f32, tag="z0", name=f"z0{li}")
        nc.scalar.activation(z0[:], pt[0:R, :], AF.Copy)
        z = apool.tile([R, SL], f32, tag="z", name=f"z{li}")
        nc.vector.tensor_tensor(z[:], z0[:], pt[R:2 * R, :], op=ALU.add)
        act = apool.tile([R, SL], bf16, tag="act", name=f"act{li}")
        nc.scalar.activation(act[:], z[:], AF.Lrelu, alpha=SLOPE, scale=scale)
        att = atp.tile([KC, 4 * R], f8, tag="att", name=f"att{li}")
        for j in range(4):
            tp = pst.tile([KC, R], bf16, tag="pst", name=f"tp{li}_{j}")
            nc.tensor.transpose(tp[:], act[:, j * KC:(j + 1) * KC], id64[:])
            nc.vector.tensor_copy(att[:, j * R:(j + 1) * R], tp[:])
        return att

    def gather(att, li):
        ag_in = dram.tile([KC, 4 * R], f8, tag=f"agin{li}")
        nc.scalar.dma_start(ag_in[:], att[:])
        ag_out = dram.tile([NC * KC, 4 * R], f8, tag=f"agout{li}",
                           addr_space="Shared")
        nc.gpsimd.collective_compute(
            "AllGather", ALU.bypass, replica_groups=rg,
            ins=[ag_in[:].opt()], outs=[ag_out[:].opt()],
        )
        engs = [nc.sync, nc.scalar, nc.gpsimd]
        chunks = []
        for r in range(NC):
            lt = lpool.tile([KC, 4 * R], f8, tag=f"lt{r}", name=f"lt{li}_{r}")
            engs[r % 3].dma_start(lt[:], ag_out[r * KC:(r + 1) * KC, :])
            for j in range(4):
                chunks.append((lt[:, j * R:(j + 1) * R], r * 4 + j))
        return chunks

    def warm_spin(count, li):
        for i in range(count):
            dpt = ps.tile([2 * R, SL], f32, tag="ps", name=f"wsp{li}_{i}")
            nc.tensor.matmul(dpt[0:R, :], xt[:, 0:R], wts[2][:, 0:SL],
                             start=True, stop=True, tile_position=(0, 0),
                             skip_group_check=True)

    # ---- L1..L4 ----
    chunks = [(xt[:, 0:R], 0), (xt[:, R:2 * R], 1)]
    for li in range(1, 5):
        pt = layer_mm(chunks, wts[li], bts[li], li)
        att = act_transpose(pt, li, 1.0 if li == 1 else 1.0 / SC)
        chunks = gather(att, li)
        if li >= 2:
            warm_spin(WSPIN, li)

    # ---- L5 (no gather) ----
    pt5 = layer_mm(chunks, wts[5], bts[5], 5)
    att5 = act_transpose(pt5, 5, 1.0 / SC)

    # ---- L6 row-split partial: z6 = a5_c^T-chunks @ W6p-rows ----
    pt6a = ps6.tile([R, SL], f32, tag="p6a")
    pt6b = ps6.tile([R, SL], f32, tag="p6b")
    for kc in range(4):
        lhs = att5[:, kc * R:(kc + 1) * R]
        nc.tensor.matmul(pt6a[:], lhs, wts[6][:, kc * OF:kc * OF + SL],
                         start=(kc == 0), stop=(kc == 3),
                         tile_position=(0, 0), skip_group_check=True)
        nc.tensor.matmul(pt6b[:], lhs, wts[6][:, kc * OF + SL:(kc + 1) * OF],
                         start=(kc == 0), stop=(kc == 3),
                         tile_position=(0, 0), skip_group_check=True)
    z6 = apool.tile([R, OF], bf16, tag="z6", bufs=1)
    nc.vector.tensor_copy(z6[:, 0:SL], pt6a[:])
    nc.scalar.activation(z6[:, SL:OF], pt6b[:], AF.Copy)
    rs_in = dram.tile([R, OF], bf16, tag="rsin")
    nc.sync.dma_start(rs_in[:], z6[:])
    rs_out = dram.tile([NC, OF], bf16, tag="rsout")
    nc.gpsimd.collective_compute(
        "ReduceScatter", ALU.add, replica_groups=rg,
        ins=[rs_in[:].opt()], outs=[rs_out[:].opt()],
    )

    # ---- tail: 8 rows on this core ----
    zza = tailp.tile([128, N], bf16, tag="zza")
    zzb = tailp.tile([128, N], bf16, tag="zzb")
    nc.sync.dma_start(
        zza[:], rs_out[0:4, :].rearrange("r (j i) -> (r j) i", i=N))
    nc.scalar.dma_start(
        zzb[:], rs_out[4:8, :].rearrange("r (j i) -> (r j) i", i=N))

    def poltile(zz, name):
        zb = tailp.tile([128, N], f32, tag=f"zb_{name}")
        nc.vector.tensor_tensor(zb[:], zz[:], bias6[:], op=ALU.add)
        sc = tailp.tile([128, N], f32, tag=f"sc_{name}")
        nc.vector.tensor_scalar_mul(sc[:], zb[:], SLOPE)
        lr = tailp.tile([128, N], f32, tag=f"lr_{name}")
        nc.vector.tensor_tensor(lr[:], zb[:], sc[:], op=ALU.max)
        sg = tailp.tile([128, N], f32, tag=f"sg_{name}")
        nc.scalar.activation(sg[:], lr[:], AF.Sigmoid)
        pol = tailp.tile([128, N], bf16, tag=f"pol_{name}")
        nc.vector.tensor_scalar(pol[:], sg[:], mac[:, 0:1], mac[:, 1:2],
                                op0=ALU.mult, op1=ALU.add)
        return pol

    pola = poltile(zza, "a")
    polb = poltile(zzb, "b")
    for rl in range(4):
        s = slice(rl * 32, (rl + 1) * 32)
        nc.vector.tensor_copy(x1a[s, s], pola[s, :])
        nc.scalar.activation(x1b[s, s], polb[s, :], AF.Copy)

    def streamT(x, name):
        y = tailp.tile([128, 128], bf16, tag=f"y_{name}")
        nc.vector.transpose(y[:], x[:])
        return y

    y1a = streamT(x1a, "1a")
    y1b = streamT(x1b, "1b")

    def sq(x, y, name, want_y=True):
        px = tps.tile([128, 128], f32, tag="tps", name=f"px{name}")
        nc.tensor.matmul(px[:], y[:], x[:], start=True, stop=True)
        x2 = tailp.tile([128, 128], bf16, tag=f"x_{name}")
        nc.vector.tensor_copy(x2[:], px[:])
        if not want_y:
            return x2, None
        py = tps.tile([128, 128], f32, tag="tps", name=f"py{name}")
        nc.tensor.matmul(py[:], x[:], y[:], start=True, stop=True)
        y2 = tailp.tile([128, 128], bf16, tag=f"y_{name}")
        nc.scalar.activation(y2[:], py[:], AF.Copy)
        return x2, y2

    x2a, y2a = sq(x1a, y1a, "2a")
    x2b, y2b = sq(x1b, y1b, "2b")
    x4a, y4a = sq(x2a, y2a, "4a")
    x4b, y4b = sq(x2b, y2b, "4b")
    x8a, _ = sq(x4a, y4a, "8a", want_y=False)
    x8b, _ = sq(x4b, y4b, "8b", want_y=False)

    # bv as a ROW: ones^T @ X8 = column sums of X8 = rowsums of M^8
    bva_ps = tps.tile([1, 128], f32, tag="tps", name="bvaps")
    nc.tensor.matmul(bva_ps[:], ones128[:], x8a[:], start=True, stop=True)
    bvb_ps = tps.tile([1, 128], f32, tag="tps", name="bvbps")
    nc.tensor.matmul(bvb_ps[:], ones128[:], x8b[:], start=True, stop=True)
    bvr = tailp.tile([1, 2 * KC], f32, tag="bvr")
    nc.vector.tensor_copy(bvr[0:1, 0:128], bva_ps[:])
    nc.scalar.activation(bvr[0:1, 128:256], bvb_ps[:], AF.Copy)

    # single-partition delta-coefficient tail
    recipE = tailp.tile([1, 2 * KC], f32, tag="recipE")
    nc.vector.reciprocal(recipE[:], bvr[:])
    tmp = tailp.tile([1, 2 * KC], f32, tag="tmp")
    nc.vector.tensor_tensor(tmp[:], bvr[:], dmf[:], op=ALU.mult)
    srcv = tailp.tile([1, 8], f32, tag="srcv")
    nc.vector.reduce_sum(srcv[:], tmp[:].rearrange("o (r i) -> o r i", i=N),
                         axis=AX.X)
    rd = tailp.tile([1, 8], f32, tag="rd")
    nc.vector.reciprocal(rd[:], srcv[:])
    coefS = tailp.tile([1, 8], f32, tag="coefS")
    nc.vector.tensor_tensor(coefS[:], w01r[:], rd[:], op=ALU.mult)
    tmp2 = tailp.tile([1, 2 * KC], f32, tag="tmp2")
    nc.vector.tensor_tensor(tmp2[:], ttf[:], recipE[:], op=ALU.mult)
    c23 = tailp.tile([1, 8], f32, tag="c23")
    nc.vector.reduce_sum(c23[:], tmp2[:].rearrange("o (r i) -> o r i", i=N),
                         axis=AX.X)
    wsS = tailp.tile([1, 2 * KC], f32, tag="wsS")
    nc.vector.tensor_tensor(
        wsS[:].rearrange("o (r i) -> o r i", i=N),
        bvr[:].rearrange("o (r i) -> o r i", i=N),
        coefS[:].unsqueeze(2).broadcast_to((1, 8, N)),
        op=ALU.mult,
    )
    pdS = tailp.tile([1, N], f32, tag="pdS")
    nc.vector.reduce_sum(pdS[:], wsS[:].rearrange("o (r i) -> o i r", i=N),
                         axis=AX.X)
    wsT = tailp.tile([1, 2 * KC], f32, tag="wsT")
    nc.vector.tensor_tensor(
        wsT[:].rearrange("o (r i) -> o r i", i=N),
        bvr[:].rearrange("o (r i) -> o r i", i=N),
        c23[:].unsqueeze(2).broadcast_to((1, 8, N)),
        op=ALU.mult,
    )
    pdT = tailp.tile([1, N], f32, tag="pdT")
    nc.vector.reduce_sum(pdT[:], wsT[:].rearrange("o (r i) -> o i r", i=N),
                         axis=AX.X)
    pdSb = tailp.tile([1, B * N], f32, tag="pdSb")
    nc.vector.tensor_tensor(
        pdSb[:].rearrange("o (b j) -> o b j", j=N),
        selsf[:].rearrange("o (b j) -> o b j", j=N),
        pdS[:].unsqueeze(1).broadcast_to((1, B, N)),
        op=ALU.mult,
    )
    pdflat = tailp.tile([1, B * N], f32, tag="pdflat")
    nc.vector.tensor_tensor(
        pdflat[:].rearrange("o (b j) -> o b j", j=N),
        seltf[:].rearrange("o (b j) -> o b j", j=N),
        pdT[:].unsqueeze(1).broadcast_to((1, B, N)),
        op=ALU.mult,
    )
    nc.vector.tensor_tensor(pdflat[:], pdflat[:], pdSb[:], op=ALU.add)

    # final gather of per-core partial deltas + sum on every core
    agf_in = dram.tile([1, B * N], f32, tag="agfin")
    nc.scalar.dma_start(agf_in[:], pdflat[:])
    agf_out = dram.tile([NC, B * N], f32, tag="agfout", addr_space="Shared")
    nc.gpsimd.collective_compute(
        "AllGather", ALU.bypass, replica_groups=rg,
        ins=[agf_in[:].opt()], outs=[agf_out[:].opt()],
    )
    pdall = tailp.tile([1, NC * B * N], f32, tag="pdall")
    nc.scalar.dma_start(
        pdall[:], agf_out[:].rearrange("k f -> (k f)").unsqueeze(0))
    osb = tailp.tile([1, B * N], f32, tag="osb")
    nc.vector.reduce_sum(
        osb[:], pdall[:].rearrange("o (k f) -> o f k", f=B * N), axis=AX.X)
    nc.sync.dma_start(
        aps["out"][:].rearrange("b j -> (b j)").unsqueeze(0), osb[:])
    es.close()


def build():
    import concourse.bacc as bacc
    import concourse.mybir as mybir
    import concourse.tile as tile

    f32 = mybir.dt.float32
    bf16 = mybir.dt.bfloat16
    f8 = mybir.dt.float8e4
    nc = bacc.Bacc("TRN2", target_bir_lowering=False, debug=False, num_devices=NC)
    shapes = {
        "XT": ([KC, 2 * R], bf16),
        "W1": ([KC, 2 * SL], bf16), "b1": ([SL], bf16),
        "W2": ([KC, 32 * SL], f8), "b2": ([SL], bf16),
        "W3": ([KC, 32 * SL], f8), "b3": ([SL], bf16),
        "W4": ([KC, 32 * SL], f8), "b4": ([SL], bf16),
        "W5": ([KC, 32 * SL], f8), "b5": ([SL], bf16),
        "W6": ([KC, 4 * OF], bf16),
        "BIAS6": ([128, N], f32), "MAC": ([128, 2], f32),
        "DMF": ([1, 2 * KC], f32), "TTF": ([1, 2 * KC], f32),
        "W01R": ([1, 8], f32),
        "SELSF": ([1, B * N], f32), "SELTF": ([1, B * N], f32),
        "ID64": ([64, 64], bf16),
    }
    aps = {
        k: nc.dram_tensor(k, v[0], v[1], kind="ExternalInput").ap()
        for k, v in shapes.items()
    }
    aps["out"] = nc.dram_tensor("out", [B, N], f32, kind="ExternalOutput").ap()
    with tile.TileContext(nc) as tc:
        _build_body(nc, tc, tile, mybir, aps)
    nc.compile()
    return nc


def prep_in_maps(inputs):
    import ml_dtypes
    f = np.float32
    bf = ml_dtypes.bfloat16
    f8 = ml_dtypes.float8_e4m3fn
    E = np.asarray(inputs["batch_node_embeddings"], f)   # (B,N,D)
    T = np.asarray(inputs["batch_Ts"], f)                # (B,N,N)
    mult = np.asarray(inputs["mult_const_batch"], f).reshape(-1)[0]
    add = np.asarray(inputs["add_const_batch"], f).reshape(-1)[0]
    S = np.transpose(E, (1, 0, 2))                       # (N,B,D)
    G0 = np.concatenate([S[:, 0], S[:, 1]], axis=-1)     # (32, 2D)
    G1 = np.concatenate([S[:, 2], S[:, 3]], axis=-1)
    rows = np.concatenate([G0, G1], axis=0)              # (64, 256)

    def packk(Wslice):
        nk = Wslice.shape[0] // KC
        return np.ascontiguousarray(
            Wslice.reshape(nk, KC, -1).transpose(1, 0, 2).reshape(KC, -1)
        )

    perm = np.arange(OF).reshape(N, N).T.reshape(-1)     # perm[j*32+i] = i*32+j
    W6perm = np.asarray(inputs["W6"], f)[:, perm]
    b6p = np.asarray(inputs["b6"], f)[perm]

    common = {
        "XT": packk(rows.T).astype(bf),
        "BIAS6": np.ascontiguousarray(np.tile(b6p.reshape(N, N), (4, 1))),
        "MAC": np.ascontiguousarray(
            np.stack([np.full(128, mult, f), np.full(128, add, f)], axis=1)
        ),
        "ID64": np.eye(64, dtype=bf),
    }
    in_maps = []
    for c in range(NC):
        m = dict(common)
        W1 = np.asarray(inputs["W1"], f)
        b1 = np.asarray(inputs["b1"], f)
        m["W1"] = packk(W1[:, c * SL:(c + 1) * SL]).astype(bf)
        m["b1"] = np.ascontiguousarray(b1[c * SL:(c + 1) * SL]).astype(bf)
        for li in range(2, 6):
            W = np.asarray(inputs[f"W{li}"], f)
            b = np.asarray(inputs[f"b{li}"], f)
            m[f"W{li}"] = (packk(W[:, c * SL:(c + 1) * SL]) * SC).astype(f8)
            m[f"b{li}"] = np.ascontiguousarray(
                b[c * SL:(c + 1) * SL] * SC).astype(bf)
        m["W6"] = packk(W6perm[c * SL:(c + 1) * SL, :]).astype(bf)
        bS = 0 if c < 4 else 1
        bT = 2 if c < 4 else 3
        dmf = np.zeros((1, 2 * KC), f)
        ttf = np.zeros((1, 2 * KC), f)
        w01r = np.zeros((1, 8), f)
        for rl in range(8):
            s = (8 * c + rl) % N
            dmf[0, rl * N + s] = 1.0
            ttf[0, rl * N:(rl + 1) * N] = T[bT][:, s]
            w01r[0, rl] = T[bS][s, :].sum()
        selsf = np.zeros((1, B * N), f)
        seltf = np.zeros((1, B * N), f)
        selsf[0, bS * N:(bS + 1) * N] = 1.0
        seltf[0, bT * N:(bT + 1) * N] = 1.0
        m["DMF"] = dmf
        m["TTF"] = ttf
        m["W01R"] = w01r
        m["SELSF"] = selsf
        m["SELTF"] = seltf
        in_maps.append(m)
    return in_maps


def kernel(**inputs):
    global _COMPILED, LAST_RESULTS
    from concourse import bass_utils

    if _COMPILED is None:
        _COMPILED = build()
    in_maps = prep_in_maps(inputs)
    res = bass_utils.run_bass_kernel_spmd(
        _COMPILED, in_maps, core_ids=list(range(NC))
    )
    LAST_RESULTS = res
    return np.asarray(res.results[0]["out"], np.float32)
